# revision 8
# baseline (speedup 1.0000x reference)
"""MoE layer kernel for 8x TRN2 NeuronCores (Bass/Tile).

Math (reference):
    w      = softmax(x @ gate_W + gate_b, axis=-1)[:E]          # [E, F]
    W_eff  = einsum('ef,edf->df', w, expert_W)                  # [D, F]
    b_eff  = einsum('ef,ef->f',  w, expert_b)                   # [F]
    out    = x @ W_eff + b_eff                                  # [N, F]

Sharding: F-parallel across 8 cores (each core owns 128 f-columns).
  - gate_W/gate_b are column-rolled per core so the shard is columns 0:128
    (the softmax row-sum is order invariant, so rolling columns is harmless).
  - expert_W[:, :, shard], expert_b[:, shard] per core.
  - Each core computes out[:, shard].T as [128, 4096]; the host transposes
    and concatenates.

Device algorithm per core:
  1. Load x in [128, 256] chunks, build xT [2][128, 4096] via PE transposes.
  2. Gate GEMM (tokens 0..1023) + bias + exp (+row-sum via accum_out) +
     normalize -> w_norm in [128p, 8, 128] (e = a*128 + p) layout.
  3. b_eff via elementwise mult + ones-matvec (PSUM-accumulated).
  4. W_eff shard via 2048 PE matvec matmuls: stream expert_W tiles
     [e=128, d=128, f=128] (contiguous 64KB/partition DMA), for each f:
     psum[:, dh*128+f] += We_tile[:, :, f].T @ w_norm[:, ebi, f].
     PSUM accumulates over the 8 e-blocks.
  5. out^T = W_eff_shard^T @ x^T + b_eff (bias folded in as a k=1 matmul
     against a ones row), chunked DMA out.

NOTE on structure: this walrus build rejects any instruction carrying more
than ONE semaphore wait ("Too many sync wait commands"). The kernel is
therefore arranged so that every instruction has unsatisfied deps from at
most one source processor: feeder copies are pinned to DVE, psum->sbuf
output copies to ACT, and cheap "touch" matmuls / ACT copies absorb ticks
one source at a time (engine-observed-tick transitivity makes later waits
on the same source free). SBUF pools are never closed mid-kernel (pool
closure creates multi-source released-zone deps); only PSUM pools close,
with PE touches absorbing the old zone's reader ticks first.
"""

import numpy as np

N, D, E, F = 4096, 256, 1024, 1024
NCORES = 8
FSH = F // NCORES  # 128 f-columns per core
P = 128

_CACHE = {}
LAST_RESULT = None


def _split_multi_waits(nc):
    """Walrus in this toolchain rejects instructions carrying more than one
    semaphore wait ("Too many sync wait commands"). Split every multi-wait
    instruction: hoist all but the last wait onto standalone EventSemaphore
    instructions inserted just before it on the same engine queue (the same
    primitive Tile's own barriers use). Semantics are identical — the
    engine's sequencer performs the waits sequentially either way."""
    import concourse.mybir as mybir

    n = 0
    for fn in nc.m.functions:
        for bb in fn.blocks:
            out = []
            changed = False
            for ins in bb.instructions:
                si = ins.sync_info
                if si is not None and si.on_wait and len(si.on_wait) > 1:
                    waits = list(si.on_wait)
                    for w in waits[:-1]:
                        es = mybir.InstEventSemaphore(
                            name=f"wsplit_{n}",
                            engine=ins.engine,
                            sync_info=mybir.SyncInfo(
                                on_wait=[w], on_update=[]),
                        )
                        out.append(es)
                        n += 1
                    ins.sync_info = mybir.SyncInfo(
                        on_wait=[waits[-1]], on_update=list(si.on_update))
                    changed = True
                out.append(ins)
            if changed:
                bb.instructions = out
    return n


def _build_bass():
    import concourse.bass as bass
    import concourse.mybir as mybir
    from concourse.masks import make_identity
    from concourse.tile import TileContext

    f32 = mybir.dt.float32
    AF = mybir.ActivationFunctionType

    nc = bass.Bass(trn_type="TRN2", name="moe_fshard",
                   dynamic_dma_scratch_size=4096)

    x_d = nc.dram_tensor("x", [N, D], f32, kind="ExternalInput")
    gw_d = nc.dram_tensor("gw", [D, F], f32, kind="ExternalInput")
    gb_d = nc.dram_tensor("gb", [1, F], f32, kind="ExternalInput")
    ew_d = nc.dram_tensor("ew", [E, D, FSH], f32, kind="ExternalInput")
    eb_d = nc.dram_tensor("eb", [E, FSH], f32, kind="ExternalInput")
    out_d = nc.dram_tensor("outT", [FSH, N], f32, kind="ExternalOutput")

    EBLK = E // P      # 8 expert blocks (and 8 gate token tiles)
    TTILE = N // P     # 32 token tiles for the x transpose
    DH = D // P        # 2 halves of d

    with TileContext(nc) as tc:
        with tc.tile_pool(name="persist", bufs=1) as persist, \
             tc.tile_pool(name="xcp", bufs=2) as xcp, \
             tc.tile_pool(name="wep", bufs=2) as wep, \
             tc.tile_pool(name="dummyp", bufs=1, space="PSUM") as dummyp:

            # ---- persistent SBUF tiles (flat; nothing released mid-kernel)
            # smalls packs the tiny constants to avoid per-tile pad waste:
            #  [:,0:128] identity; [:,128:129] ones_col; row0 129:257
            #  ones_r128; row0 257:769 ones_r512; row0 769:897 beff_row;
            #  row0 897:898 act_scratch; row0 1024:2048 gate_b
            smalls = persist.tile([P, 2048], f32)
            ident = smalls[:, 0:128]
            ones_col = smalls[:, 128:129]
            ones_r128 = smalls[0:1, 129:257]
            ones_r512 = smalls[0:1, 257:769]
            beff_row = smalls[0:1, 769:897]
            act_scr = smalls[0:1, 897:898]
            gb_sb = smalls[0:1, 1024:2048]

            make_identity(nc, ident)
            nc.vector.memset(smalls[:, 128:129], 1.0)
            nc.vector.memset(smalls[0:1, 129:769], 1.0)

            xT = persist.tile([P, DH, N], f32)          # 32KB/part
            wnorm = persist.tile([P, EBLK, FSH], f32)   # 4KB/part
            weff = persist.tile([P, DH * FSH], f32)     # 1KB/part
            # scr packs DVE scratch: rsum [:,0:1], rcp [:,1:2],
            # tmpb bufs at [:,128:256] and [:,256:384]
            scr = persist.tile([P, 384], f32)
            rsum = scr[:, 0:1]
            rcp = scr[:, 1:2]
            expsc = persist.tile([P, F], f32)           # 4KB/part, bufs=1
            outT_sb = persist.tile([P, N], f32)         # 16KB/part

            gw_sb = persist.tile([P, DH, F], f32)       # 8KB/part
            eb_sb = persist.tile([P, EBLK, FSH], f32)   # 4KB/part

            dummy = dummyp.tile([1, 1], f32)

            # ---- input DMAs (no deps)
            nc.sync.dma_start(
                out=gw_sb[:], in_=gw_d.rearrange("(h p) f -> p h f", p=P))
            nc.sync.dma_start(out=gb_sb, in_=gb_d[:, :])
            nc.sync.dma_start(
                out=eb_sb[:], in_=eb_d.rearrange("(a p) f -> p a f", p=P))

            # ================= Phase 1a: x -> xT ======================
            with tc.tile_pool(name="tpsum", bufs=2, space="PSUM") as tpsum, \
                 tc.tile_pool(name="gpsum", bufs=2, space="PSUM") as gpsum, \
                 tc.tile_pool(name="bpsum", bufs=1, space="PSUM") as bpsum:

                # PE touch: absorb gpsimd tick (identity) before transposes
                nc.tensor.matmul(dummy[:], ident[:, 0:1], ident[:, 0:1],
                                 start=True, stop=True)

                for a in range(TTILE):
                    xc = xcp.tile([P, D], f32, tag="xc")
                    nc.sync.dma_start(out=xc[:],
                                      in_=x_d[a * P:(a + 1) * P, :])
                    # PE touch: absorb this chunk's DMA tick so each
                    # transpose carries at most the pt-release wait
                    nc.tensor.matmul(dummy[:], xc[:, 0:1], xc[:, 0:1],
                                     start=True, stop=True)
                    for dh in range(DH):
                        pt = tpsum.tile([P, P], f32, tag="pt")
                        nc.tensor.transpose(
                            pt[:], xc[:, dh * P:(dh + 1) * P], ident)
                        nc.vector.tensor_copy(
                            xT[:, dh, a * P:(a + 1) * P], pt[:])

                # ============= Phase 1b: gate + softmax ===============
                expsc_ap = expsc[:]
                for a in range(EBLK):
                    lp = gpsum.tile([P, F], f32, tag="lp")
                    for half in range(2):
                        sl = slice(half * 512, (half + 1) * 512)
                        nc.tensor.matmul(lp[:, sl],
                                         xT[:, 0, a * P:(a + 1) * P],
                                         gw_sb[:, 0, sl],
                                         start=True, stop=False)
                        nc.tensor.matmul(lp[:, sl],
                                         xT[:, 1, a * P:(a + 1) * P],
                                         gw_sb[:, 1, sl],
                                         start=False, stop=False)
                        nc.tensor.matmul(lp[:, sl], ones_r128, gb_sb[:, sl],
                                         start=False, stop=True)
                    if a >= 1:
                        # ACT touch: absorb ts_mul(a-1)'s DVE tick so
                        # exp(a) (WAR on expsc/rsum) has only the PE wait
                        nc.scalar.copy(act_scr, wnorm[0:1, a - 1, 0:1])
                    nc.scalar.activation(expsc_ap, lp[:], AF.Exp,
                                         accum_out=rsum)
                    nc.vector.reciprocal(rcp, rsum)
                    nc.vector.tensor_scalar_mul(
                        wnorm[:, a, :], expsc[:, 0:FSH], rcp)

                # ============= Phase 1c: b_eff ========================
                bps = bpsum.tile([1, FSH], f32)
                for a in range(EBLK):
                    tmpb = scr[:, 128 + (a % 2) * 128:256 + (a % 2) * 128]
                    nc.vector.tensor_mul(tmpb, wnorm[:, a, :],
                                         eb_sb[:, a, :])
                    nc.tensor.matmul(bps[:], ones_col, tmpb,
                                     start=(a == 0), stop=(a == EBLK - 1))
                nc.vector.tensor_copy(beff_row, bps[:])

            # ================= Phase 2: W_eff =========================
            with tc.tile_pool(name="wpsum", bufs=1, space="PSUM") as wpsum:
                # PE touches: absorb the released psum zone's reader ticks
                # (DVE: beff copy was the last DVE psum read; ACT: exp(7)
                # was the last ACT psum read) before the wps matmuls.
                nc.tensor.matmul(dummy[:], beff_row[0:1, 0:1],
                                 beff_row[0:1, 0:1], start=True, stop=True)
                nc.tensor.matmul(dummy[:], expsc[:, 0:1], expsc[:, 0:1],
                                 start=True, stop=True)

                # One psum slot per (ebi, dh, f); every matmul is its own
                # single-MM group (start=True clears has_written bits for
                # the WHOLE bank, so interleaved accumulation groups would
                # silently drop contributions — sum the 8 e-block planes on
                # DVE instead).
                wps = wpsum.tile([P, EBLK * DH * FSH], f32)  # 8KB = 4 banks
                for ebi in range(EBLK):
                    for dh in range(DH):
                        wet = wep.tile([P, P, FSH], f32, tag="we")
                        nc.sync.dma_start(
                            out=wet[:],
                            in_=ew_d[ebi * P:(ebi + 1) * P,
                                     dh * P:(dh + 1) * P, :])
                        for f in range(FSH):
                            slot = ebi * DH * FSH + dh * FSH + f
                            nc.tensor.matmul(
                                wps[:, slot:slot + 1],
                                wet[:, :, f:f + 1],
                                wnorm[:, ebi, f:f + 1],
                                start=True, stop=True,
                                skip_group_check=True)
                nc.vector.tensor_copy(weff[:], wps[:, 0:DH * FSH])
                for ebi in range(1, EBLK):
                    sl = slice(ebi * DH * FSH, (ebi + 1) * DH * FSH)
                    nc.vector.tensor_add(weff[:], wps[:, sl], weff[:])

            # ============== Phase 3: outT = W_eff.T @ xT + b ==========
            with tc.tile_pool(name="fpsum", bufs=2, space="PSUM") as fpsum:
                for ch in range(N // 512):
                    sl = slice(ch * 512, (ch + 1) * 512)
                    ps = fpsum.tile([P, 512], f32, tag="fp")
                    nc.tensor.matmul(ps[:], weff[:, 0:FSH], xT[:, 0, sl],
                                     start=True, stop=False)
                    nc.tensor.matmul(ps[:], weff[:, FSH:2 * FSH],
                                     xT[:, 1, sl],
                                     start=False, stop=False)
                    nc.tensor.matmul(ps[:], beff_row, ones_r512,
                                     start=False, stop=True)
                    nc.scalar.copy(outT_sb[:, sl], ps[:])
                    nc.sync.dma_start(out=out_d[:, sl], in_=outT_sb[:, sl])

    _split_multi_waits(nc)
    return nc


def kernel(x, gate_W, gate_b, expert_W, expert_b, _trace=False):
    global LAST_RESULT
    from concourse.bass_utils import run_bass_kernel_spmd

    x = np.ascontiguousarray(np.asarray(x, dtype=np.float32))
    gate_W = np.asarray(gate_W, dtype=np.float32)
    gate_b = np.asarray(gate_b, dtype=np.float32).reshape(1, F)
    expert_W = np.asarray(expert_W, dtype=np.float32)
    expert_b = np.asarray(expert_b, dtype=np.float32)

    if "nc" not in _CACHE:
        _CACHE["nc"] = _build_bass()
    nc = _CACHE["nc"]

    in_maps = []
    for c in range(NCORES):
        sh = slice(c * FSH, (c + 1) * FSH)
        in_maps.append({
            "x": x,
            # roll shard columns to the front; softmax row-sum is invariant
            "gw": np.ascontiguousarray(np.roll(gate_W, -c * FSH, axis=1)),
            "gb": np.ascontiguousarray(np.roll(gate_b, -c * FSH, axis=1)),
            "ew": np.ascontiguousarray(expert_W[:, :, sh]),
            "eb": np.ascontiguousarray(expert_b[:, sh]),
        })

    res = run_bass_kernel_spmd(
        nc, in_maps, core_ids=list(range(NCORES)), trace=_trace,
    )
    LAST_RESULT = res

    out = np.empty([N, F], dtype=np.float32)
    for c in range(NCORES):
        out[:, c * FSH:(c + 1) * FSH] = res.results[c]["outT"].T
    return out
